# revision 7
# baseline (speedup 1.0000x reference)
"""Bahdanau-style attention kernel for Trainium2, data-parallel over batch
across 8 NeuronCores.

Reference computation (per batch b):
    e_proj = enc[b] @ We.T            # [S, D]   (We = W[:, 512:], [D, E])
    energy = tanh(e_proj + hidden[b] @ Wh.T + bias)
    scores = energy @ v               # [S]
    attn   = softmax(scores)          # [1, S]

Shapes: B=32, S=2048, E=1024, D=512.  Each core handles 4 batches.

Device-side design (per core), v2 — PE-density focused:
  - Work unit u = (batch bi, S-half h): 8 units of [1024 s, 1024 e].
  - Per unit: DMA f32 -> DVE cast f16 -> PE transpose (e to partitions)
    -> PE main matmul psum[d128, 1024] (2 banks, one tanh w/ fused
    per-partition bias) -> PE matvec with v -> softmax per batch.
  - Unit u+1's transposes are interleaved BETWEEN unit u's matmul
    chains so the PE HAM clock never sees a >3.4us stretch without
    real matmuls (transpose-mode doesn't count as PE-busy for HAM).
  - W/hidden prep is done in f16 right after a DVE cast; batch-0 enc
    processing no longer serializes behind it.
  - All PSUM->SBUF copies on DVE (ACT copies are 2-9x slower); ScalarE
    does only tanh/exp.  Output DMAs go on the gpsimd queue so the
    sync queue's enc prefetch stream is never blocked.
"""

import numpy as np

B, S, E, D = 32, 2048, 1024, 512
N_CORES = 8
BP = B // N_CORES   # batches per core = 4
SU = 1024           # s-range per unit
N_U = BP * S // SU  # 8 units
N_ST = SU // 128    # 8 s-subtiles per unit
N_EC = E // 128     # 8 e-chunks
N_DP = D // 128     # 4 d-chunks
N_KC = D // 128     # 4 k-chunks (hidden proj contraction)

_CACHE = {}


def _build():
    from contextlib import ExitStack

    import concourse.bass as bass
    import concourse.tile as tile
    from concourse import bacc, mybir
    from concourse.masks import make_identity

    F32 = mybir.dt.float32
    F16 = mybir.dt.float16
    AF = mybir.ActivationFunctionType
    AX = mybir.AxisListType

    nc = bacc.Bacc("TRN2", target_bir_lowering=False, debug=False,
                   num_devices=N_CORES)

    hid_d = nc.dram_tensor("hidden", [BP, D], F32, kind="ExternalInput").ap()
    enc_d = nc.dram_tensor("enc", [BP, S, E], F32, kind="ExternalInput").ap()
    w_d = nc.dram_tensor("W", [D, D + E], F32, kind="ExternalInput").ap()
    b_d = nc.dram_tensor("b", [D], F32, kind="ExternalInput").ap()
    v_d = nc.dram_tensor("v", [D], F32, kind="ExternalInput").ap()
    out_d = nc.dram_tensor("out", [BP, S], F32, kind="ExternalOutput").ap()

    with tile.TileContext(nc) as tc, ExitStack() as ctx:
        consts = ctx.enter_context(tc.tile_pool(name="consts", bufs=1))
        enc_pool = ctx.enter_context(tc.tile_pool(name="enc", bufs=2))
        enc16_pool = ctx.enter_context(tc.tile_pool(name="enc16", bufs=2))
        enct_pool = ctx.enter_context(tc.tile_pool(name="enct", bufs=2))
        en_pool = ctx.enter_context(tc.tile_pool(name="energy", bufs=2))
        small = ctx.enter_context(tc.tile_pool(name="small", bufs=2))
        sm1 = ctx.enter_context(tc.tile_pool(name="sm1", bufs=2))
        ptr_ps = ctx.enter_context(tc.tile_pool(name="ptr", bufs=2, space="PSUM"))
        pe_ps = ctx.enter_context(tc.tile_pool(name="pe", bufs=2, space="PSUM"))
        sc_ps = ctx.enter_context(tc.tile_pool(name="sc", bufs=2, space="PSUM"))

        # ---- input DMAs (sync queue, in emission order) ----
        # W is loaded in two [128, 2, 1536] chunks sharing the enc32 chunk
        # tag (keeps per-partition SBUF for staging at 3x16 KB).
        w_sb = [None, None]
        for c in range(2):
            w_sb[c] = enc_pool.tile([128, 2, D + E], F32, tag="enc32",
                                    name=f"w_sb_{c}")
            nc.sync.dma_start(
                out=w_sb[c],
                in_=w_d[c * 256:(c + 1) * 256, :].rearrange(
                    "(dp p) q -> p dp q", p=128
                ),
            )
        hid_sb = consts.tile([BP, D], F32)
        nc.sync.dma_start(out=hid_sb, in_=hid_d)
        b_sb4 = consts.tile([N_DP, 128], F32)
        nc.sync.dma_start(out=b_sb4, in_=b_d.rearrange("(dp q) -> dp q", q=128))
        v_sb4 = consts.tile([N_DP, 128], F32)
        nc.sync.dma_start(out=v_sb4, in_=v_d.rearrange("(dp q) -> dp q", q=128))

        enc32 = [[None, None] for _ in range(N_U)]
        enc16 = [None] * N_U

        def emit_enc_dma(u):
            bi, h = divmod(u, 2)
            for c in range(2):
                enc32[u][c] = enc_pool.tile(
                    [128, N_ST // 2, E], F32, tag="enc32", name=f"enc32_{u}_{c}"
                )
                nc.sync.dma_start(
                    out=enc32[u][c],
                    in_=enc_d[bi, h * SU + c * 512: h * SU + (c + 1) * 512, :]
                    .rearrange("(st p) e -> p st e", p=128),
                )

        def emit_cast(u):
            enc16[u] = enc16_pool.tile([128, N_ST, E], F16, tag="enc16", name=f"enc16_{u}")
            for j in range(4):
                nc.vector.tensor_copy(
                    enc16[u][:, 2 * j:2 * j + 2, :],
                    enc32[u][j // 2][:, 2 * (j % 2):2 * (j % 2) + 2, :],
                )

        # prefetch the first two units
        emit_enc_dma(0)
        emit_enc_dma(1)

        identity16 = consts.tile([128, 128], F16)
        make_identity(nc, identity16)
        identity32 = consts.tile([8, 8], F32)
        make_identity(nc, identity32)

        # preload activation tables early (overlaps with DMAs)
        warm = consts.tile([1, 1], F32)
        nc.vector.memset(warm, 0.0)
        nc.scalar.activation(warm, warm, AF.Tanh)
        nc.scalar.activation(warm, warm, AF.Exp)

        # ---- weight prep (all f16); w16 shares the enc16 slot cycle ----
        w16 = enc16_pool.tile([128, N_DP, D + E], F16, tag="enc16", name="w16")
        for j in range(2):
            nc.vector.tensor_copy(w16[:, 2 * j:2 * j + 2, :], w_sb[j])
        hid16 = consts.tile([BP, D], F16)
        nc.vector.tensor_copy(hid16, hid_sb)
        v16 = consts.tile([N_DP, 128], F16)
        nc.vector.tensor_copy(v16, v_sb4)

        # WeT [e, ec, d] f16
        wet_sb = consts.tile([128, N_EC, D], F16)
        for ec in range(N_EC):
            pt = ptr_ps.tile([128, D], F16, tag="ptr")
            with tc.tile_critical():
                for dp in range(N_DP):
                    nc.tensor.matmul(
                        pt[:, dp * 128:(dp + 1) * 128],
                        w16[:, dp, D + ec * 128:D + (ec + 1) * 128],
                        identity16, is_transpose=True,
                        start=(dp == 0), stop=(dp == N_DP - 1),
                    )
            nc.vector.tensor_copy(wet_sb[:, ec, :], pt)

        # WhT [k, kc, d] f16
        wht_sb = consts.tile([128, N_KC, D], F16)
        for kc in range(N_KC):
            pt = ptr_ps.tile([128, D], F16, tag="ptr")
            with tc.tile_critical():
                for dp in range(N_DP):
                    nc.tensor.matmul(
                        pt[:, dp * 128:(dp + 1) * 128],
                        w16[:, dp, kc * 128:(kc + 1) * 128],
                        identity16, is_transpose=True,
                        start=(dp == 0), stop=(dp == N_DP - 1),
                    )
            nc.vector.tensor_copy(wht_sb[:, kc, :], pt)

        # hidden^T [k, kc, b] f16
        hidt_sb = consts.tile([128, N_KC, BP], F16)
        for kc in range(N_KC):
            pt = sc_ps.tile([128, 16], F16, tag="sc")
            nc.tensor.matmul(
                pt[:, 0:BP], hid16[:, kc * 128:(kc + 1) * 128],
                identity16[0:BP, 0:BP], is_transpose=True,
            )
            nc.vector.tensor_copy(hidt_sb[:, kc, :], pt[:, 0:BP])

        # b^T [d, dp] f32 ; v^T [d, dp] f16
        bt_sb = consts.tile([128, N_DP], F32)
        pt = sc_ps.tile([128, 16], F32, tag="sc")
        nc.tensor.matmul(pt[:, 0:N_DP], b_sb4, identity32[0:N_DP, 0:N_DP],
                         is_transpose=True)
        nc.vector.tensor_copy(bt_sb, pt[:, 0:N_DP])

        vt_sb = consts.tile([128, N_DP], F16)
        pt = sc_ps.tile([128, 16], F16, tag="sc")
        nc.tensor.matmul(pt[:, 0:N_DP], v16, identity16[0:N_DP, 0:N_DP],
                         is_transpose=True)
        nc.vector.tensor_copy(vt_sb, pt[:, 0:N_DP])

        # h_projT + bias -> hbT [d, dp, b] f32
        hbt_sb = consts.tile([128, N_DP, BP], F32)
        for dp in range(N_DP):
            ph = sc_ps.tile([128, 16], F32, tag="sc")
            for kc in range(N_KC):
                nc.tensor.matmul(
                    ph[:, 0:BP],
                    wht_sb[:, kc, dp * 128:(dp + 1) * 128],
                    hidt_sb[:, kc, :],
                    start=(kc == 0), stop=(kc == N_KC - 1),
                )
            nc.vector.tensor_scalar_add(
                hbt_sb[:, dp, :], ph[:, 0:BP], bt_sb[:, dp:dp + 1]
            )

        # ---- per-unit transpose: enc16[u] -> enct[u] [e, ec, s(1024)] ----
        enct = [None] * N_U

        def emit_transpose_group(u, g):
            # group g = (q, p): s-half q (4 subtiles), e-chunk pair p
            q, p = divmod(g, N_EC // 2)
            if g == 0:
                enct[u] = enct_pool.tile([128, N_EC, SU], F16, tag="enct", name=f"enct_{u}")
            pt = ptr_ps.tile([128, 1024], F16, tag="ptr")
            with tc.tile_critical():
                for j in range(2):
                    ec = 2 * p + j
                    for t in range(4):
                        st = 4 * q + t
                        nc.tensor.matmul(
                            pt[:, j * 512 + t * 128: j * 512 + (t + 1) * 128],
                            enc16[u][:, st, ec * 128:(ec + 1) * 128],
                            identity16, is_transpose=True,
                            start=(j == 0 and t == 0),
                            stop=(j == 1 and t == 3),
                        )
            nc.vector.tensor_copy(
                enct[u][:, 2 * p:2 * p + 2, q * 512:(q + 1) * 512],
                pt.rearrange("p (j f) -> p j f", j=2),
            )

        # bootstrap: unit 0 fully transposed up front (chases its DMA)
        emit_cast(0)
        for g in range(8):
            emit_transpose_group(0, g)

        # ---- main pipeline over units ----
        scores = [None] * BP
        for u in range(N_U):
            bi, h = divmod(u, 2)
            if u + 2 < N_U:
                emit_enc_dma(u + 2)
            if u + 1 < N_U:
                emit_cast(u + 1)
            if h == 0:
                scores[bi] = small.tile([1, S], F32, tag="scores", name=f"scores_{bi}")

            energy = en_pool.tile([128, N_DP, SU], F16, tag="energy")
            for dp in range(N_DP):
                pe = pe_ps.tile([128, SU], F32, tag="pe")
                for q in range(2):
                    for ec in range(N_EC):
                        nc.tensor.matmul(
                            pe[:, q * 512:(q + 1) * 512],
                            wet_sb[:, ec, dp * 128:(dp + 1) * 128],
                            enct[u][:, ec, q * 512:(q + 1) * 512],
                            start=(ec == 0), stop=(ec == N_EC - 1),
                        )
                nc.scalar.activation(
                    energy[:, dp, :], pe, AF.Tanh,
                    bias=hbt_sb[:, dp, bi:bi + 1], scale=1.0,
                )
                # interleave next unit's transposes between matmul chains
                # (keeps HAM warm; transpose-mode doesn't count as PE-busy)
                if u + 1 < N_U:
                    if dp == 1:
                        emit_transpose_group(u + 1, 0)
                        emit_transpose_group(u + 1, 1)
                    elif dp == 2:
                        emit_transpose_group(u + 1, 2)
                        emit_transpose_group(u + 1, 3)
                    elif dp == 3:
                        for g in range(4, 8):
                            emit_transpose_group(u + 1, g)

            # scores: v . energy  (contraction over d on partitions)
            for q in range(2):
                psc = sc_ps.tile([1, 512], F32, tag="sc")
                for dp in range(N_DP):
                    nc.tensor.matmul(
                        psc, vt_sb[:, dp:dp + 1],
                        energy[:, dp, q * 512:(q + 1) * 512],
                        start=(dp == 0), stop=(dp == N_DP - 1),
                    )
                nc.vector.tensor_copy(
                    scores[bi][:, h * SU + q * 512: h * SU + (q + 1) * 512], psc
                )

            if h == 1:
                # softmax over S for batch bi; scores are O(+-35) so exp in
                # f32 needs no max subtraction (exp arg < 88 always holds)
                prob = sm1.tile([1, S], F32, tag="pa", name=f"prob_{bi}")
                nc.scalar.activation(prob, scores[bi], AF.Exp)
                ssum = small.tile([1, 1], F32, tag="ssum")
                nc.vector.reduce_sum(ssum, prob, axis=AX.X)
                rtot = small.tile([1, 1], F32, tag="rtot")
                nc.vector.reciprocal(rtot, ssum)
                attn = sm1.tile([1, S], F32, tag="pa", name=f"attn_{bi}")
                nc.vector.tensor_scalar_mul(attn, prob, rtot)
                # gpsimd DMA queue: don't block the sync queue's enc stream
                nc.gpsimd.dma_start(out=out_d[bi], in_=attn)

    nc.compile()
    return nc


def _get_nc():
    if "nc" not in _CACHE:
        _CACHE["nc"] = _build()
    return _CACHE["nc"]


def kernel(hidden, encoder_outputs, W, b, v):
    from concourse.bass_utils import run_bass_kernel_spmd

    nc = _get_nc()
    hidden = np.ascontiguousarray(hidden, dtype=np.float32)
    encoder_outputs = np.ascontiguousarray(encoder_outputs, dtype=np.float32)
    W = np.ascontiguousarray(W, dtype=np.float32)
    b = np.ascontiguousarray(b, dtype=np.float32)
    v = np.ascontiguousarray(v, dtype=np.float32)

    in_maps = [
        {
            "hidden": hidden[c * BP:(c + 1) * BP],
            "enc": encoder_outputs[c * BP:(c + 1) * BP],
            "W": W,
            "b": b,
            "v": v,
        }
        for c in range(N_CORES)
    ]
    r = run_bass_kernel_spmd(nc, in_maps, list(range(N_CORES)))
    out = np.concatenate([r.results[c]["out"] for c in range(N_CORES)], axis=0)
    return out[:, None, :].astype(np.float32)
